# revision 3
# baseline (speedup 1.0000x reference)
"""Trainium2 Bass kernel for a cross-attention layer (data-parallel over 8 cores).

Math (per batch b):
  G[i,t]      = sum_h img[i,h] * txt[t,h]            (shared Gram matrix)
  call1 (txt queries img):   score1 = G  (layout [img, txt])
     relu, l1-normalize over txt, softmax(*9) over img
     w1 = attn1 @ img ; out1 = relu(w1 @ W_txt^T + b_txt) ; attn_img = attn1
  call2 (img queries txt):   score2 = G^T (layout [txt, img])
     relu, l1-normalize over img, softmax(*9) over txt
     out2 = relu((attn2 @ txt) @ W_img^T + b_img)
     (reassociated: txtW = txt @ W_img^T first, saves 0.67 GFLOP/batch)

Layout strategy: keep the softmax normalizations as per-partition scalars only
(no partition broadcasts).  Unnormalized exp matrices feed the matmuls; the
1/rowsum factors are applied as per-partition scales on the outputs, with the
bias injected through an extra K=1 matmul row (dsm[q] * b[o], then the whole
PSUM tile is scaled by 1/dsm[q]).

float32r note: tiles consumed by regular (non-transpose) matmuls must be
written pre-rounded to fp32r by their producer instruction (BIR verifier
rule), so those tiles are allocated with dtype float32r; DMA-filled ones get
the DRAM AP bitcast.
"""

import numpy as np

import concourse.bass as bass
import concourse.mybir as mybir
import concourse.tile as tile
from concourse import bacc
from concourse.bass_utils import run_bass_kernel_spmd
from concourse.masks import make_identity

N_CORES = 8
B, LT, LI, H = 128, 256, 576, 1024
BPC = B // N_CORES
SMOOTH = 9.0
EPS = 1e-8

F32 = mybir.dt.float32
F32R = mybir.dt.float32r
AF = mybir.ActivationFunctionType
AX = mybir.AxisListType

USE_F32R = True  # matmuls in fp32r (full PE rate); transposes stay fp32-exact
MMDT = F32R if USE_F32R else F32  # dtype for tiles consumed by regular matmuls

KH = H // 128           # 8 h-tiles
KT_T = LT // 128        # 2 txt partition tiles
# img partition tiles: (offset, width)
IMG_TILES = [(0, 128), (128, 128), (256, 128), (384, 128), (512, 64)]
KT_I = len(IMG_TILES)
NCHUNK = [(0, 512), (512, 512)]          # output H chunks (one PSUM bank each)
G_NCHUNK = [(0, 512), (512, 64)]         # img chunks for the Gram matmul


def _dma(ap):
    # DRAM-side bitcast so byte-copy DMAs can target float32r tiles
    return ap.bitcast(F32R) if USE_F32R else ap


def build(bpc=BPC):
    nc = bacc.Bacc("TRN2", target_bir_lowering=False, debug=False)

    txt = nc.dram_tensor("txt_embed", [bpc, LT, H], F32, kind="ExternalInput").ap()
    img = nc.dram_tensor("img_embed", [bpc, LI, H], F32, kind="ExternalInput").ap()
    w_txt = nc.dram_tensor("W_txt", [H, H], F32, kind="ExternalInput").ap()
    b_txt = nc.dram_tensor("b_txt", [H], F32, kind="ExternalInput").ap()
    w_img = nc.dram_tensor("W_img", [H, H], F32, kind="ExternalInput").ap()
    b_img = nc.dram_tensor("b_img", [H], F32, kind="ExternalInput").ap()
    out1 = nc.dram_tensor("txt_attn_output", [bpc, LT, H], F32, kind="ExternalOutput").ap()
    out2 = nc.dram_tensor("img_attn_output", [bpc, LI, H], F32, kind="ExternalOutput").ap()
    attn1 = nc.dram_tensor("attn_img", [bpc, LT, LI], F32, kind="ExternalOutput").ap()
    attn2 = nc.dram_tensor("attn_txt", [bpc, LI, LT], F32, kind="ExternalOutput").ap()

    with tile.TileContext(nc) as tc:
        _body(tc, bpc, txt, img, w_txt, b_txt, w_img, b_img, out1, out2, attn1, attn2)
    nc.compile()
    return nc


def _body(tc, bpc, txt, img, w_txt, b_txt, w_img, b_img, out1, out2, attn1, attn2):
    nc = tc.nc
    import contextlib

    ctx = contextlib.ExitStack()
    with ctx:
        p_const = ctx.enter_context(tc.tile_pool(name="const", bufs=1))
        p_txt = ctx.enter_context(tc.tile_pool(name="txtin", bufs=2))
        p_bigin = ctx.enter_context(tc.tile_pool(name="bigin", bufs=1))
        p_txtT = ctx.enter_context(tc.tile_pool(name="txtT", bufs=1))
        p_imgT = ctx.enter_context(tc.tile_pool(name="imgT", bufs=1))
        p_rgt = ctx.enter_context(tc.tile_pool(name="rgt", bufs=1))
        p_e1T = ctx.enter_context(tc.tile_pool(name="e1T", bufs=1))
        p_E2 = ctx.enter_context(tc.tile_pool(name="E2", bufs=1))
        p_w1 = ctx.enter_context(tc.tile_pool(name="w1uT", bufs=1))
        p_txtW = ctx.enter_context(tc.tile_pool(name="txtW", bufs=1))
        p_out = ctx.enter_context(tc.tile_pool(name="outs", bufs=4))
        p_attn1 = ctx.enter_context(tc.tile_pool(name="attn1", bufs=2))
        p_attn2 = ctx.enter_context(tc.tile_pool(name="attn2", bufs=2))
        p_vec = ctx.enter_context(tc.tile_pool(name="vec", bufs=2))
        p_row = ctx.enter_context(tc.tile_pool(name="rows", bufs=2))

        ps_big = ctx.enter_context(tc.tile_pool(name="ps_big", bufs=2, space="PSUM"))
        ps_small = ctx.enter_context(tc.tile_pool(name="ps_small", bufs=2, space="PSUM"))
        ps_z = ctx.enter_context(tc.tile_pool(name="ps_z", bufs=2, space="PSUM"))

        # ---- constants ----
        ident = p_const.tile([128, 128], F32)
        make_identity(nc, ident[:])

        bt_row = p_const.tile([1, H], MMDT)
        nc.sync.dma_start(bt_row[:], _dma(b_txt.unsqueeze(0)))
        bi_row = p_const.tile([1, H], MMDT)
        nc.sync.dma_start(bi_row[:], _dma(b_img.unsqueeze(0)))

        # ---- transpose the two Linear weights once: WT[h, o] = W[o, h] ----
        wts = []
        for wi, wdram in enumerate((w_txt, w_img)):
            WT = p_const.tile([128, KH, H], MMDT, tag=f"WT{wi}")
            for quarter in range(4):  # stage 2 o-tiles at a time
                wstage = p_bigin.tile([128, 2, H], F32, tag="bigin")
                nc.sync.dma_start(
                    wstage[:],
                    wdram[quarter * 256:(quarter + 1) * 256, :].rearrange(
                        "(ko p) h -> p ko h", p=128
                    ),
                )
                for kh in range(KH):
                    wps = ps_small.tile([128, 256], F32, tag="small")
                    for j in range(2):
                        nc.tensor.transpose(
                            wps[:, j * 128:(j + 1) * 128],
                            wstage[:, j, kh * 128:(kh + 1) * 128],
                            ident[:],
                        )
                    eng = nc.vector.tensor_copy if kh % 2 == 0 else nc.scalar.copy
                    eng(WT[:, kh, quarter * 256:(quarter + 1) * 256], wps[:])
            wts.append(WT)
        W_txtT, W_imgT = wts

        # ---- per-batch pipeline ----
        for b in range(bpc):
            # load inputs
            txt_sb = p_txt.tile([128, KT_T, H], F32, tag="txtin")
            nc.sync.dma_start(txt_sb[:], txt[b].rearrange("(kt p) h -> p kt h", p=128))
            img_sb = p_bigin.tile([128, KT_I, H], MMDT, tag="bigin")
            nc.sync.dma_start(
                img_sb[:, 0:4, :],
                _dma(img[b, 0:512, :].rearrange("(kt p) h -> p kt h", p=128)),
            )
            nc.sync.dma_start(img_sb[0:64, 4, :], _dma(img[b, 512:576, :]))

            # txtT[h, t] and imgT[h, i]
            txtT = p_txtT.tile([128, KH, LT], MMDT, tag="txtT")
            for kh in range(KH):
                pst = ps_small.tile([128, LT], F32, tag="small")
                for t in range(KT_T):
                    nc.tensor.transpose(
                        pst[:, t * 128:(t + 1) * 128],
                        txt_sb[:, t, kh * 128:(kh + 1) * 128],
                        ident[:],
                    )
                nc.vector.tensor_copy(txtT[:, kh, :], pst[:])
            imgT = p_imgT.tile([128, KH, LI], MMDT, tag="imgT")
            for kh in range(KH):
                psb = ps_big.tile([128, LI], F32, tag="big")
                for c, (p0, pw) in enumerate(IMG_TILES):
                    nc.tensor.transpose(
                        psb[:, p0:p0 + pw],
                        img_sb[0:pw, c, kh * 128:(kh + 1) * 128].bitcast(F32),
                        ident[0:pw, 0:pw],
                    )
                nc.scalar.copy(imgT[:, kh, :], psb[:])

            # G^T[t, i] (txt on partitions), fused relu + row-sums over img (= d2)
            relu_gT = p_rgt.tile([128, KT_T, LI], F32, tag="rgt")
            d2 = p_vec.tile([128, KT_T], F32, tag="d2")
            scale2 = p_vec.tile([128, KT_T], F32, tag="scale2")
            for t in range(KT_T):
                g_ps = ps_big.tile([128, LI], F32, tag="big")
                for n0, nw in G_NCHUNK:
                    for kh in range(KH):
                        nc.tensor.matmul(
                            g_ps[:, n0:n0 + nw],
                            txtT[:, kh, t * 128:(t + 1) * 128],
                            imgT[:, kh, n0:n0 + nw],
                            start=(kh == 0),
                            stop=(kh == KH - 1),
                        )
                nc.scalar.activation(
                    relu_gT[:, t, :], g_ps[:], AF.Relu, accum_out=d2[:, t:t + 1]
                )
            # scale2 = SMOOTH / (d2 + EPS)   (call2's l1 norm is over img = free dim)
            nc.vector.tensor_scalar_add(scale2[:], d2[:], EPS)
            nc.vector.reciprocal(scale2[:], scale2[:])
            nc.vector.tensor_scalar_mul(scale2[:], scale2[:], SMOOTH)

            # E2[t, i] = exp(scale2 * relu_gT)  (unnormalized attn2^T)
            E2 = p_E2.tile([128, KT_T, LI], MMDT, tag="E2")
            for t in range(KT_T):
                nc.scalar.activation(
                    E2[:, t, :], relu_gT[:, t, :], AF.Exp, scale=scale2[:, t:t + 1]
                )

            # relu_g[i, t] via PE transpose; d1 = rowsums over txt; e1T = exp
            e1T = p_e1T.tile([128, KT_I, LT], MMDT, tag="e1T")
            d1 = p_vec.tile([128, KT_I], F32, tag="d1")
            scale1 = p_vec.tile([128, KT_I], F32, tag="scale1")
            for c, (p0, pw) in enumerate(IMG_TILES):
                rg_ps = ps_small.tile([128, LT], F32, tag="small")
                for t in range(KT_T):
                    nc.tensor.transpose(
                        rg_ps[0:pw, t * 128:(t + 1) * 128],
                        relu_gT[:, t, p0:p0 + pw],
                        ident[:],
                    )
                nc.vector.reduce_sum(d1[0:pw, c:c + 1], rg_ps[0:pw, :], axis=AX.X)
                nc.vector.tensor_scalar_add(scale1[0:pw, c:c + 1], d1[0:pw, c:c + 1], EPS)
                nc.vector.reciprocal(scale1[0:pw, c:c + 1], scale1[0:pw, c:c + 1])
                nc.vector.tensor_scalar_mul(
                    scale1[0:pw, c:c + 1], scale1[0:pw, c:c + 1], SMOOTH
                )
                nc.scalar.activation(
                    e1T[0:pw, c, :], rg_ps[0:pw, :], AF.Exp, scale=scale1[0:pw, c:c + 1]
                )

            # w1uT[h, t] = img^T @ e1T   (unnormalized attn-weighted img, transposed)
            w1uT = p_w1.tile([128, KH, LT], MMDT, tag="w1uT")
            for kh in range(KH):
                w_ps = ps_small.tile([128, LT], F32, tag="small")
                for c, (p0, pw) in enumerate(IMG_TILES):
                    nc.tensor.matmul(
                        w_ps[:],
                        img_sb[0:pw, c, kh * 128:(kh + 1) * 128],
                        e1T[0:pw, c, :],
                        start=(c == 0),
                        stop=(c == KT_I - 1),
                    )
                nc.vector.tensor_copy(w1uT[:, kh, :], w_ps[:])

            # attn1 = e1 / dsm1 (e1 = transpose of e1T); dsm1_row for the bias matmul
            dsm1_col = p_vec.tile([128, KT_T], F32, tag="dsm1c")
            rs1 = p_vec.tile([128, KT_T], F32, tag="rs1")
            dsm1_row = p_row.tile([1, LT], MMDT, tag="dsm1r")
            for t in range(KT_T):
                e1_ps = ps_big.tile([128, LI], F32, tag="big")
                for c, (p0, pw) in enumerate(IMG_TILES):
                    nc.tensor.transpose(
                        e1_ps[:, p0:p0 + pw],
                        e1T[0:pw, c, t * 128:(t + 1) * 128].bitcast(F32),
                        ident[0:pw, 0:pw],
                    )
                nc.vector.reduce_sum(dsm1_col[:, t:t + 1], e1_ps[:], axis=AX.X)
                nc.vector.reciprocal(rs1[:, t:t + 1], dsm1_col[:, t:t + 1])
                a1 = p_attn1.tile([128, LI], F32, tag="attn1")
                nc.vector.tensor_scalar_mul(a1[:], e1_ps[:], rs1[:, t:t + 1])
                nc.sync.dma_start(attn1[b, t * 128:(t + 1) * 128, :], a1[:])
                dr_ps = ps_small.tile([1, 128], F32, tag="small")
                nc.tensor.transpose(dr_ps[:], dsm1_col[:, t:t + 1], ident[:])
                nc.vector.tensor_copy(dsm1_row[0:1, t * 128:(t + 1) * 128], dr_ps[:])

            # out1 = relu(rs1 * (w1uT^T @ W_txtT + dsm1*b_txt))
            for t in range(KT_T):
                for n0, nw in NCHUNK:
                    z_ps = ps_z.tile([128, 512], F32, tag="z")
                    for kh in range(KH):
                        nc.tensor.matmul(
                            z_ps[:],
                            w1uT[:, kh, t * 128:(t + 1) * 128],
                            W_txtT[:, kh, n0:n0 + nw],
                            start=(kh == 0),
                            stop=False,
                        )
                    nc.tensor.matmul(
                        z_ps[:],
                        dsm1_row[0:1, t * 128:(t + 1) * 128],
                        bt_row[0:1, n0:n0 + nw],
                        start=False,
                        stop=True,
                    )
                    o = p_out.tile([128, 512], F32, tag="outs")
                    nc.scalar.activation(o[:], z_ps[:], AF.Relu, scale=rs1[:, t:t + 1])
                    nc.sync.dma_start(out1[b, t * 128:(t + 1) * 128, n0:n0 + nw], o[:])

            # txtW[t, o] = txt @ W_img^T  (reassociated call2 linear)
            txtW = p_txtW.tile([128, KT_T, H], MMDT, tag="txtW")
            for t in range(KT_T):
                for n0, nw in NCHUNK:
                    tw_ps = ps_z.tile([128, 512], F32, tag="z")
                    for kh in range(KH):
                        nc.tensor.matmul(
                            tw_ps[:],
                            txtT[:, kh, t * 128:(t + 1) * 128],
                            W_imgT[:, kh, n0:n0 + nw],
                            start=(kh == 0),
                            stop=(kh == KH - 1),
                        )
                    nc.vector.tensor_copy(txtW[:, t, n0:n0 + nw], tw_ps[:])

            # attn2 = E2^T / dsm2; dsm2_row for bias matmul
            dsm2_col = p_vec.tile([128, KT_I], F32, tag="dsm2c")
            rs2 = p_vec.tile([128, KT_I], F32, tag="rs2")
            dsm2_row = p_row.tile([1, LI], MMDT, tag="dsm2r")
            for c, (p0, pw) in enumerate(IMG_TILES):
                e2t_ps = ps_small.tile([128, LT], F32, tag="small")
                for t in range(KT_T):
                    nc.tensor.transpose(
                        e2t_ps[0:pw, t * 128:(t + 1) * 128],
                        E2[:, t, p0:p0 + pw].bitcast(F32),
                        ident[:],
                    )
                nc.vector.reduce_sum(dsm2_col[0:pw, c:c + 1], e2t_ps[0:pw, :], axis=AX.X)
                nc.vector.reciprocal(rs2[0:pw, c:c + 1], dsm2_col[0:pw, c:c + 1])
                a2 = p_attn2.tile([128, LT], F32, tag="attn2")
                nc.vector.tensor_scalar_mul(a2[0:pw, :], e2t_ps[0:pw, :], rs2[0:pw, c:c + 1])
                nc.sync.dma_start(attn2[b, p0:p0 + pw, :], a2[0:pw, :])
                dr2_ps = ps_small.tile([1, 128], F32, tag="small")
                nc.tensor.transpose(
                    dr2_ps[0:1, 0:pw], dsm2_col[0:pw, c:c + 1], ident[0:pw, 0:pw]
                )
                nc.vector.tensor_copy(dsm2_row[0:1, p0:p0 + pw], dr2_ps[0:1, 0:pw])

            # out2 = relu(rs2 * (E2^T-matmul txtW + dsm2*b_img))
            for c, (p0, pw) in enumerate(IMG_TILES):
                for n0, nw in NCHUNK:
                    z2_ps = ps_z.tile([128, 512], F32, tag="z")
                    for t in range(KT_T):
                        nc.tensor.matmul(
                            z2_ps[0:pw, :],
                            E2[:, t, p0:p0 + pw],
                            txtW[:, t, n0:n0 + nw],
                            start=(t == 0),
                            stop=False,
                        )
                    nc.tensor.matmul(
                        z2_ps[0:pw, :],
                        dsm2_row[0:1, p0:p0 + pw],
                        bi_row[0:1, n0:n0 + nw],
                        start=False,
                        stop=True,
                    )
                    o2 = p_out.tile([128, 512], F32, tag="outs")
                    nc.scalar.activation(
                        o2[0:pw, :], z2_ps[0:pw, :], AF.Relu, scale=rs2[0:pw, c:c + 1]
                    )
                    nc.sync.dma_start(out2[b, p0:p0 + pw, n0:n0 + nw], o2[0:pw, :])


_NC_CACHE = {}


def get_nc(bpc=BPC):
    if bpc not in _NC_CACHE:
        _NC_CACHE[bpc] = build(bpc)
    return _NC_CACHE[bpc]


def kernel(**inputs):
    txt = np.ascontiguousarray(np.asarray(inputs["txt_embed"], dtype=np.float32))
    img = np.ascontiguousarray(np.asarray(inputs["img_embed"], dtype=np.float32))
    W_txt = np.ascontiguousarray(np.asarray(inputs["W_txt"], dtype=np.float32))
    b_txt = np.ascontiguousarray(np.asarray(inputs["b_txt"], dtype=np.float32))
    W_img = np.ascontiguousarray(np.asarray(inputs["W_img"], dtype=np.float32))
    b_img = np.ascontiguousarray(np.asarray(inputs["b_img"], dtype=np.float32))

    nb = txt.shape[0]
    bpc = nb // N_CORES
    nc = get_nc(bpc)

    in_maps = [
        {
            "txt_embed": txt[c * bpc:(c + 1) * bpc],
            "img_embed": img[c * bpc:(c + 1) * bpc],
            "W_txt": W_txt,
            "b_txt": b_txt,
            "W_img": W_img,
            "b_img": b_img,
        }
        for c in range(N_CORES)
    ]
    res = run_bass_kernel_spmd(nc, in_maps, list(range(N_CORES)))
    out1 = np.concatenate([res.results[c]["txt_attn_output"] for c in range(N_CORES)], axis=0)
    out2 = np.concatenate([res.results[c]["img_attn_output"] for c in range(N_CORES)], axis=0)
    a_img = np.concatenate([res.results[c]["attn_img"] for c in range(N_CORES)], axis=0)
    a_txt = np.concatenate([res.results[c]["attn_txt"] for c in range(N_CORES)], axis=0)
    return (out1, out2, a_img, a_txt)
